# revision 1
# baseline (speedup 1.0000x reference)
"""Trainium2 Bass kernel for nn_Conditional_encoding (2-layer conditional LSTM encoder).

Data-parallel over 8 NeuronCores: batch 1024 -> 128 per core.

Per-core design (batch-on-partition layout, B=128):
  - Embedding table padded to [V, 64] f32, col 63 = 1.0 (bias slot).
  - Per step: indirect-DMA gather of 128 token rows -> x tile [128, 64].
  - Recurrent stationary built by two PE "transpose" matmuls (lhsT=data,
    rhs=identity) into one PSUM tile: x.T -> rows 0:64, h.T -> rows 64:128,
    then one ACT copy PSUM->SBUF.
  - One gate matmul: [K=128] stationary stkT x rhs W [128, 256] -> PSUM
    gates [batch, 256] with col blocks [i | f | 2*g | o] (g block of W
    pre-doubled on host so a single Sigmoid activation computes
    sigma(i), sigma(f), p=sigma(2g), sigma(o); tanh(g) = 2p-1).
  - DVE/GPSIMD elementwise: tg=2p-1 (gpsimd tensor_scalar), m=sigma(i)*tg
    (DVE), v=sigma(f)*c (gpsimd), c'=m+v (DVE), h=sigma(o)*tanh(c')
    (ACT tanh + gpsimd mul).
  - take_along_axis gathers: host-precomputed per-step uint8 masks +
    copy_predicated accumulation (exact, one DVE op per step).
  - Final MLP via PE transposes + 2 matmuls.
"""

import sys

sys.path.insert(0, "/opt/trn_rl_repo")

import numpy as np

import concourse.bass as bass
import concourse.mybir as mybir
from concourse import tile
from concourse.bass_utils import run_bass_kernel_spmd

B, T1, T2, V, D, H = 1024, 256, 256, 32004, 50, 64
NCORES = 8
BS = B // NCORES  # 128 batch rows per core
E = 64  # padded embedding width (D=50 data + zeros + ones col at 63)
F32 = mybir.dt.float32
I32 = mybir.dt.int32
U8 = mybir.dt.uint8


def _split_excess_waits(nc, max_waits=1):
    """Walrus CTRL lowering rejects multi-sem-wait instructions in this
    toolchain; move excess waits onto same-engine NOPs inserted before."""
    n_fixed = 0
    for f in nc.m.functions:
        for bb in f.blocks:
            insts = bb.instructions
            i = 0
            while i < len(insts):
                inst = insts[i]
                si = getattr(inst, "sync_info", None)
                if si is not None and si.on_wait and len(si.on_wait) > max_waits:
                    waits = list(si.on_wait)
                    si.on_wait = waits[-max_waits:]
                    excess = waits[:-max_waits]
                    pos = i
                    for j in range(0, len(excess), max_waits):
                        nop = mybir.InstNoOp(
                            name=f"{inst.name}-waitfix-{j}",
                            sync_info=mybir.SyncInfo(
                                on_wait=excess[j : j + max_waits], on_update=[]
                            ),
                            bass_nofuse=True,
                            engine=inst.engine,
                        )
                        insts.insert(pos, nop)
                        pos += 1
                        i += 1
                    n_fixed += 1
                i += 1
    return n_fixed


def _lstm_step(nc, pools, sent, tbl, ident, w, hc, t, acc, mask, mask_cols):
    """Emit one LSTM step. hc: persistent [128, 128] tile, cols 0:64 = c,
    64:128 = h. acc/mask: gather accumulator + per-step mask columns."""
    ts = tile.TilePool  # noqa (doc only)
    g = pools["gx"].tile([BS, E], F32, tag="gx")
    nc.gpsimd.indirect_dma_start(
        out=g[:],
        out_offset=None,
        in_=tbl[:],
        in_offset=bass.IndirectOffsetOnAxis(ap=sent[:, t : t + 1], axis=0),
    )
    pT = pools["pT"].tile([128, BS], F32, tag="pT")
    nc.tensor.matmul(pT[0:64, :], g[:], ident[:])
    nc.tensor.matmul(pT[64:128, :], hc[:, 64:128], ident[:])
    sT = pools["sT"].tile([128, BS], F32, tag="sT")
    nc.scalar.copy(sT[:, 0:80], pT[:, 0:80])
    nc.vector.tensor_copy(sT[:, 80:128], pT[:, 80:128])
    pg = pools["pg"].tile([BS, 256], F32, tag="pg")
    nc.tensor.matmul(pg[:], sT[:], w[:])
    sg = pools["sg"].tile([BS, 256], F32, tag="sg")
    nc.scalar.activation(sg[:], pg[:], mybir.ActivationFunctionType.Sigmoid)
    tg = pools["tg"].tile([BS, 64], F32, tag="tg")
    nc.gpsimd.tensor_scalar(
        tg[:], sg[:, 128:192], 2.0, -1.0, mybir.AluOpType.mult, mybir.AluOpType.add
    )
    m = pools["m"].tile([BS, 64], F32, tag="m")
    nc.vector.tensor_tensor(m[:], sg[:, 0:64], tg[:], mybir.AluOpType.mult)
    v = pools["v"].tile([BS, 64], F32, tag="v")
    nc.gpsimd.tensor_tensor(v[:], sg[:, 64:128], hc[:, 0:64], mybir.AluOpType.mult)
    nc.vector.tensor_tensor(hc[:, 0:64], m[:], v[:], mybir.AluOpType.add)
    tc_ = pools["tc"].tile([BS, 64], F32, tag="tc")
    nc.scalar.activation(tc_[:], hc[:, 0:64], mybir.ActivationFunctionType.Tanh)
    nc.gpsimd.tensor_tensor(
        hc[:, 64:128], sg[:, 192:256], tc_[:], mybir.AluOpType.mult
    )
    # masked gather accumulation: acc = where(mask_t, hc_slice, acc)
    mc = mask_cols
    nc.vector.copy_predicated(
        acc[:, 0:mc], mask[:, t * mc : (t + 1) * mc], hc[:, 128 - mc : 128]
    )


def build_program(t1=T1, t2=T2):
    nc = bass.Bass()
    tbl = nc.declare_dram_parameter("tbl", [V, E], F32, isOutput=False)
    s1 = nc.declare_dram_parameter("s1", [BS, t1], I32, isOutput=False)
    s2 = nc.declare_dram_parameter("s2", [BS, t2], I32, isOutput=False)
    w1d = nc.declare_dram_parameter("w1", [128, 256], F32, isOutput=False)
    w2d = nc.declare_dram_parameter("w2", [128, 256], F32, isOutput=False)
    m1d = nc.declare_dram_parameter("m1", [BS, t1 * 128], U8, isOutput=False)
    m2d = nc.declare_dram_parameter("m2", [BS, t2 * 64], U8, isOutput=False)
    idd = nc.declare_dram_parameter("ident", [128, 128], F32, isOutput=False)
    wm1d = nc.declare_dram_parameter("wm1", [65, 128], F32, isOutput=False)
    wl2d = nc.declare_dram_parameter("wl2t", [128, 4], F32, isOutput=False)
    bl2d = nc.declare_dram_parameter("bl2", [4, 1], F32, isOutput=False)
    out = nc.declare_dram_parameter("out", [4, BS], F32, isOutput=True)

    with tile.TileContext(nc) as tc:
        with (
            tc.tile_pool(name="const", bufs=1) as cpool,
            tc.tile_pool(name="state", bufs=1) as spool,
            tc.tile_pool(name="gx", bufs=6) as gx,
            tc.tile_pool(name="pT", bufs=2, space="PSUM") as pT,
            tc.tile_pool(name="pg", bufs=2, space="PSUM") as pg,
            tc.tile_pool(name="sT", bufs=3) as sTp,
            tc.tile_pool(name="sg", bufs=3) as sgp,
            tc.tile_pool(name="tg", bufs=2) as tgp,
            tc.tile_pool(name="m", bufs=2) as mp,
            tc.tile_pool(name="v", bufs=2) as vp,
            tc.tile_pool(name="tc", bufs=2) as tcp,
        ):
            pools = {
                "gx": gx, "pT": pT, "pg": pg, "sT": sTp, "sg": sgp,
                "tg": tgp, "m": mp, "v": vp, "tc": tcp,
            }
            # constants
            sent1 = cpool.tile([BS, t1], I32)
            nc.sync.dma_start(out=sent1[:], in_=s1[:])
            sent2 = cpool.tile([BS, t2], I32)
            nc.sync.dma_start(out=sent2[:], in_=s2[:])
            w1 = cpool.tile([128, 256], F32)
            nc.sync.dma_start(out=w1[:], in_=w1d[:])
            w2 = cpool.tile([128, 256], F32)
            nc.sync.dma_start(out=w2[:], in_=w2d[:])
            mk1 = cpool.tile([BS, t1 * 128], U8)
            nc.sync.dma_start(out=mk1[:], in_=m1d[:])
            mk2 = cpool.tile([BS, t2 * 64], U8)
            nc.sync.dma_start(out=mk2[:], in_=m2d[:])
            ident = cpool.tile([128, 128], F32)
            nc.sync.dma_start(out=ident[:], in_=idd[:])
            wm1 = cpool.tile([65, 128], F32)
            nc.sync.dma_start(out=wm1[:], in_=wm1d[:])
            wl2t = cpool.tile([128, 4], F32)
            nc.sync.dma_start(out=wl2t[:], in_=wl2d[:])
            bl2 = cpool.tile([4, 1], F32)
            nc.sync.dma_start(out=bl2[:], in_=bl2d[:])

            # state
            hc1 = spool.tile([BS, 128], F32)  # c | h for LSTM1
            nc.vector.memset(hc1[:], 0.0)
            acc1 = spool.tile([BS, 128], F32)  # gathered c | h -> LSTM2 init
            nc.vector.memset(acc1[:], 0.0)
            acc2 = spool.tile([BS, 65], F32)  # gathered h + ones col
            nc.vector.memset(acc2[:, 0:64], 0.0)
            nc.vector.memset(acc2[:, 64:65], 1.0)

            for t in range(t1):
                _lstm_step(nc, pools, sent1, tbl, ident, w1, hc1, t, acc1, mk1, 128)
            for t in range(t2):
                _lstm_step(nc, pools, sent2, tbl, ident, w2, acc1, t, acc2, mk2, 64)

            # MLP head: z1 = tanh([gh|1] @ [Wl1.T; bl1]); out = z1 @ Wl2.T + bl2
            pmt = pT.tile([65, BS], F32, tag="pT")
            nc.tensor.matmul(pmt[:], acc2[:], ident[:])
            smt = sTp.tile([65, BS], F32, tag="sT")
            nc.scalar.copy(smt[:], pmt[:])
            pz = pg.tile([BS, 128], F32, tag="pg")
            nc.tensor.matmul(pz[:], smt[:], wm1[:])
            z1 = sgp.tile([BS, 128], F32, tag="sg")
            nc.scalar.activation(z1[:], pz[:], mybir.ActivationFunctionType.Tanh)
            pzt = pT.tile([128, BS], F32, tag="pT")
            nc.tensor.matmul(pzt[:], z1[:], ident[:])
            szt = sTp.tile([128, BS], F32, tag="sT")
            nc.scalar.copy(szt[:], pzt[:])
            po = pg.tile([4, BS], F32, tag="pg")
            nc.tensor.matmul(po[:], wl2t[:], szt[:])
            o4 = cpool.tile([4, BS], F32)
            nc.scalar.add(o4[:], po[:], bl2[:, 0:1])
            nc.sync.dma_start(out=out[:], in_=o4[:])

    _split_excess_waits(nc)
    return nc


def pack_weights(Wih, Whh, bias):
    """-> [128, 256] f32: rows 0:50 Wih.T, 63 bias, 64:128 Whh.T; g block x2."""
    w = np.zeros((128, 256), np.float32)
    w[0:D, :] = np.asarray(Wih, np.float32).T
    w[63, :] = np.asarray(bias, np.float32)
    w[64:128, :] = np.asarray(Whh, np.float32).T
    w[:, 128:192] *= 2.0  # g block: sigma(2g) trick
    return w


def build_masks(slen, t_steps, dup):
    """slen: [BS, 64] int -> [BS, t_steps * (64*dup)] uint8 one-hot over t."""
    bs = slen.shape[0]
    eq = slen[:, None, :] == np.arange(t_steps, dtype=slen.dtype)[None, :, None]
    eq = eq.astype(np.uint8)  # [BS, t, 64]
    if dup == 2:
        eq = np.concatenate([eq, eq], axis=2)  # c | h halves
    return np.ascontiguousarray(eq.reshape(bs, -1))


_prog_cache = {}


def get_program(t1=T1, t2=T2):
    key = (t1, t2)
    if key not in _prog_cache:
        _prog_cache[key] = build_program(t1, t2)
    return _prog_cache[key]


def make_in_maps(sentence1, sentence2, s1_len, s2_len, emb,
                 Wih1, Whh1, bih1, bhh1, Wih2, Whh2, bih2, bhh2,
                 Wl1, bl1, Wl2, bl2, t1=T1, t2=T2):
    emb_pad = np.zeros((V, E), np.float32)
    emb_pad[:, :D] = np.asarray(emb, np.float32)
    emb_pad[:, 63] = 1.0
    w1 = pack_weights(Wih1, Whh1, np.asarray(bih1) + np.asarray(bhh1))
    w2 = pack_weights(Wih2, Whh2, np.asarray(bih2) + np.asarray(bhh2))
    wm1 = np.zeros((65, 128), np.float32)
    wm1[0:64, :] = np.asarray(Wl1, np.float32).T
    wm1[64, :] = np.asarray(bl1, np.float32)
    wl2t = np.ascontiguousarray(np.asarray(Wl2, np.float32).T)
    bl2c = np.asarray(bl2, np.float32).reshape(4, 1)
    ident = np.eye(128, dtype=np.float32)

    s1t = np.asarray(sentence1, np.int32)
    s2t = np.asarray(sentence2, np.int32)
    l1 = np.asarray(s1_len, np.int64)[:, 0, :]  # [B, 64]
    l2 = np.asarray(s2_len, np.int64)[:, 0, :]

    in_maps = []
    for c in range(NCORES):
        sl = slice(c * BS, (c + 1) * BS)
        in_maps.append({
            "tbl": emb_pad,
            "s1": np.ascontiguousarray(s1t[sl, :t1]),
            "s2": np.ascontiguousarray(s2t[sl, :t2]),
            "w1": w1, "w2": w2,
            "m1": build_masks(l1[sl], t1, 2),
            "m2": build_masks(l2[sl], t2, 1),
            "ident": ident, "wm1": wm1, "wl2t": wl2t, "bl2": bl2c,
        })
    return in_maps


def kernel(sentence1, sentence2, s1_len, s2_len, s1_s, s2_s, emb,
           Wih1, Whh1, bih1, bhh1, Wih2, Whh2, bih2, bhh2,
           Wl1, bl1, Wl2, bl2):
    nc = get_program()
    in_maps = make_in_maps(sentence1, sentence2, s1_len, s2_len, emb,
                           Wih1, Whh1, bih1, bhh1, Wih2, Whh2, bih2, bhh2,
                           Wl1, bl1, Wl2, bl2)
    res = run_bass_kernel_spmd(nc, in_maps, list(range(NCORES)))
    out = np.zeros((B, 4), np.float32)
    for c in range(NCORES):
        out[c * BS : (c + 1) * BS, :] = res.results[c]["out"].T
    return out



# revision 7
# speedup vs baseline: 45.5156x; 45.5156x over previous
"""Trainium2 Bass kernel for nn_Conditional_encoding (2-layer conditional LSTM encoder).

Data-parallel over 8 NeuronCores: batch 1024 -> 128 per core.

Per-core design v2 ("transposed" layout, feature-on-partition):
  - Gates are computed transposed: pg[gate, batch] = W.T @ r where the
    stationary lhsT is the constant packed weight [feat 128, gate 128]
    (two matmuls per step: (f|i) and (g|o) blocks into one PSUM tile
    [128, 256]) and the moving rhs is r_t [feat 128, batch 128] with
    rows 0:50 = x_t.T, row 50 = ones (bias row), rows 64:128 = h.T.
  - h' is written by the DVE directly into rows 64:128 of the NEXT
    step's rhs tile -- no PE transpose / PSUM copy on the recurrent path.
  - x path (off the critical chain): per-step indirect DMA gather of
    128 embedding rows -> [128 tok, 64], PE transpose -> PSUM, ACT copy
    rows 0:50 into the rhs tile two steps ahead. Gathers are emitted
    LAST in each step so SWDGE generation never blocks cell math on the
    GPSIMD queue.
  - Gate blocks are ordered (i|f) and (g|o) with the g columns
    pre-doubled; two half sigmoids (the first overlaps mm_go), then
    tanh g = 2*sigma(2g)-1 is folded into a fused scalar_tensor_tensor:
      q = (2p)*sigma_i [DVE stt], v = sigma_f*c [GPSIMD], w = v - sigma_i [DVE],
      c' = q + w -> state rows 64:128; th = tanh(c') -> base 64;
      h' = sigma_o*th -> r_next[64:128]. All ops have matching input
      partition bases (c and th live at base 64).
  - take_along_axis gathers: host-precomputed per-step one-hot masks,
    copy_predicated into accumulators (c and h separately for LSTM1,
    h only for LSTM2 straight into the MLP rhs tile).
  - MLP head: [Wl1.T;bl1] and ones-row rhs, tanh, Wl2.T matmul, bias add.
"""

import sys

sys.path.insert(0, "/opt/trn_rl_repo")

import numpy as np

import concourse.bass as bass
import concourse.mybir as mybir
from concourse import tile
from concourse.bass_utils import run_bass_kernel_spmd

B, T1, T2, V, D, H = 1024, 256, 256, 32004, 50, 64
NCORES = 8
BS = B // NCORES  # 128 batch rows per core
E = 64  # padded embedding width
F32 = mybir.dt.float32
I32 = mybir.dt.int32
U8 = mybir.dt.uint8
GPF = 6  # gather prefetch distance (steps)
XPF = 2  # x-copy prefetch distance (steps)
NR = 4  # rhs ring size


def _split_excess_waits(nc, max_waits=1):
    """Walrus CTRL lowering rejects multi-sem-wait instructions in this
    toolchain; move excess waits onto same-engine NOPs inserted before."""
    n_fixed = 0
    for f in nc.m.functions:
        for bb in f.blocks:
            insts = bb.instructions
            i = 0
            while i < len(insts):
                inst = insts[i]
                si = getattr(inst, "sync_info", None)
                if si is not None and si.on_wait and len(si.on_wait) > max_waits:
                    waits = list(si.on_wait)
                    si.on_wait = waits[-max_waits:]
                    excess = waits[:-max_waits]
                    pos = i
                    for j in range(0, len(excess), max_waits):
                        nop = mybir.InstNoOp(
                            name=f"{inst.name}-waitfix-{j}",
                            sync_info=mybir.SyncInfo(
                                on_wait=excess[j : j + max_waits], on_update=[]
                            ),
                            bass_nofuse=True,
                            engine=inst.engine,
                        )
                        insts.insert(pos, nop)
                        pos += 1
                        i += 1
                    n_fixed += 1
                i += 1
    return n_fixed


def build_program(t1=T1, t2=T2):
    nc = bass.Bass()
    tbl = nc.declare_dram_parameter("tbl", [V, E], F32, isOutput=False)
    s1d = nc.declare_dram_parameter("s1", [BS, t1], I32, isOutput=False)
    s2d = nc.declare_dram_parameter("s2", [BS, t2], I32, isOutput=False)
    wfi1d = nc.declare_dram_parameter("wfi1", [128, 128], F32, isOutput=False)
    wgo1d = nc.declare_dram_parameter("wgo1", [128, 128], F32, isOutput=False)
    wfi2d = nc.declare_dram_parameter("wfi2", [128, 128], F32, isOutput=False)
    wgo2d = nc.declare_dram_parameter("wgo2", [128, 128], F32, isOutput=False)
    m1d = nc.declare_dram_parameter("m1", [128, t1 * 128], U8, isOutput=False)
    m2d = nc.declare_dram_parameter("m2", [128, t2 * 128], U8, isOutput=False)
    idd = nc.declare_dram_parameter("ident", [128, 128], F32, isOutput=False)
    wm1d = nc.declare_dram_parameter("wm1", [65, 128], F32, isOutput=False)
    wl2d = nc.declare_dram_parameter("wl2t", [128, 4], F32, isOutput=False)
    bl2d = nc.declare_dram_parameter("bl2", [4, 1], F32, isOutput=False)
    outd = nc.declare_dram_parameter("out", [4, BS], F32, isOutput=True)

    T = t1 + t2

    with tile.TileContext(nc) as tc:
        with (
            tc.tile_pool(name="const", bufs=1) as cpool,
            tc.tile_pool(name="gx", bufs=GPF + 2) as gxp,
            tc.tile_pool(name="pg", bufs=2, space="PSUM") as pgp,
            tc.tile_pool(name="pxT", bufs=2, space="PSUM") as pxp,
            tc.tile_pool(name="sg", bufs=3) as sgp,
            tc.tile_pool(name="tg", bufs=2) as tgp,
            tc.tile_pool(name="m", bufs=2) as mp,
            tc.tile_pool(name="v", bufs=2) as vp,
            tc.tile_pool(name="th", bufs=2) as thp,
        ):
            # ---- constants
            sent1 = cpool.tile([BS, t1], I32)
            nc.sync.dma_start(out=sent1[:], in_=s1d[:])
            sent2 = cpool.tile([BS, t2], I32)
            nc.sync.dma_start(out=sent2[:], in_=s2d[:])
            wfi1 = cpool.tile([128, 128], F32)
            nc.sync.dma_start(out=wfi1[:], in_=wfi1d[:])
            wgo1 = cpool.tile([128, 128], F32)
            nc.sync.dma_start(out=wgo1[:], in_=wgo1d[:])
            wfi2 = cpool.tile([128, 128], F32)
            nc.sync.dma_start(out=wfi2[:], in_=wfi2d[:])
            wgo2 = cpool.tile([128, 128], F32)
            nc.sync.dma_start(out=wgo2[:], in_=wgo2d[:])
            mk1 = cpool.tile([128, t1 * 128], U8)
            nc.sync.dma_start(out=mk1[:], in_=m1d[:])
            mk2 = cpool.tile([128, t2 * 128], U8)
            nc.sync.dma_start(out=mk2[:], in_=m2d[:])
            ident = cpool.tile([128, 128], F32)
            nc.sync.dma_start(out=ident[:], in_=idd[:])
            wm1 = cpool.tile([65, 128], F32)
            nc.sync.dma_start(out=wm1[:], in_=wm1d[:])
            wl2t = cpool.tile([128, 4], F32)
            nc.sync.dma_start(out=wl2t[:], in_=wl2d[:])
            bl2 = cpool.tile([4, 1], F32)
            nc.sync.dma_start(out=bl2[:], in_=bl2d[:])

            # ---- persistent state
            r = []
            for k in range(NR):
                rk = cpool.tile([128, 128], F32, name=f"r{k}")
                nc.vector.memset(rk[:], 0.0)
                r.append(rk)
            st = []
            for k in range(2):
                sk = cpool.tile([128, 128], F32, name=f"st{k}")
                nc.vector.memset(sk[:], 0.0)
                st.append(sk)
            acc_c = cpool.tile([64, 128], F32)
            nc.vector.memset(acc_c[:], 0.0)
            acc_h = cpool.tile([64, 128], F32)
            nc.vector.memset(acc_h[:], 0.0)
            rh = cpool.tile([65, 128], F32)  # MLP rhs; rows 0:64 = gathered h2
            nc.vector.memset(rh[:], 0.0)
            nc.vector.memset(rh[64:65, :], 1.0)

            gx = {}  # step -> gather tile

            def gather(sent, lt, t):
                g = gxp.tile([BS, E], F32, tag="gx")
                nc.gpsimd.indirect_dma_start(
                    out=g[:],
                    out_offset=None,
                    in_=tbl[:],
                    in_offset=bass.IndirectOffsetOnAxis(ap=sent[:, lt : lt + 1], axis=0),
                )
                gx[t] = g

            def xprep(t):
                """PE transpose + ACT copy of x for global step t into r[t%NR]."""
                g = gx.pop(t)
                pxT = pxp.tile([64, BS], F32, tag="pxT")
                nc.tensor.matmul(pxT[:], g[:], ident[:])
                nc.scalar.copy(r[t % NR][0:64, :], pxT[:])

            def step(t, wfi, wgo, mask, sent, lt, last_gather_lt, layer):
                rt = r[t % NR]
                rn = r[(t + 1) % NR]
                cprev = st[(t + 1) % 2]
                ccur = st[t % 2]
                pg = pgp.tile([128, 256], F32, tag="pg")
                nc.tensor.matmul(pg[:, 0:128], wfi[:], rt[:])
                nc.tensor.matmul(pg[:, 128:256], wgo[:], rt[:])
                sg = sgp.tile([128, 256], F32, tag="sg")
                nc.scalar.activation(sg[:], pg[:], mybir.ActivationFunctionType.Sigmoid)
                # c' = sf*c + si*tg, tg = 2p-1
                tg = tgp.tile([64, 128], F32, tag="q")
                nc.vector.tensor_scalar(
                    tg[:], sg[0:64, 128:256], 2.0, -1.0,
                    mybir.AluOpType.mult, mybir.AluOpType.add,
                )
                vt = vp.tile([64, 128], F32, tag="v")
                nc.vector.tensor_tensor(
                    vt[:], sg[64:128, 0:128], cprev[64:128, :], mybir.AluOpType.mult
                )
                mt = mp.tile([64, 128], F32, tag="w")
                nc.vector.tensor_tensor(
                    mt[:], sg[0:64, 0:128], tg[:], mybir.AluOpType.mult
                )
                nc.vector.tensor_tensor(
                    ccur[64:128, :], mt[:], vt[:], mybir.AluOpType.add
                )
                th = thp.tile([128, 128], F32, tag="th")
                nc.scalar.activation(
                    th[64:128, :], ccur[64:128, :], mybir.ActivationFunctionType.Tanh
                )
                nc.vector.tensor_tensor(
                    rn[64:128, :], sg[64:128, 128:256], th[64:128, :],
                    mybir.AluOpType.mult,
                )
                blk = slice(lt * 128, (lt + 1) * 128)
                if layer == 1:
                    nc.vector.copy_predicated(
                        acc_c[:], mask[64:128, blk], ccur[64:128, :]
                    )
                    nc.vector.copy_predicated(
                        acc_h[:], mask[64:128, blk], rn[64:128, :]
                    )
                else:
                    nc.vector.copy_predicated(
                        rh[0:64, :], mask[64:128, blk], rn[64:128, :]
                    )
                # off-chain x path for future steps
                if lt + XPF < last_gather_lt + 1:
                    xprep(t + XPF)
                if lt + GPF <= last_gather_lt:
                    gather(sent, lt + GPF, t + GPF)

            # ---- layer 1 prologue
            for d in range(min(GPF, t1)):
                gather(sent1, d, d)
            for d in range(min(XPF, t1)):
                xprep(d)
            # ---- layer 1
            for lt in range(t1):
                step(lt, wfi1, wgo1, mk1, sent1, lt, t1 - 1, 1)

            # ---- transition to layer 2
            nc.vector.tensor_copy(r[t1 % NR][64:128, :], acc_h[:])
            nc.vector.tensor_copy(st[(t1 + 1) % 2][64:128, :], acc_c[:])
            for d in range(min(GPF, t2)):
                gather(sent2, d, t1 + d)
            for d in range(min(XPF, t2)):
                xprep(t1 + d)
            # ---- layer 2
            for lt in range(t2):
                step(t1 + lt, wfi2, wgo2, mk2, sent2, lt, t2 - 1, 2)

            # ---- MLP head
            pz = pgp.tile([128, 128], F32, tag="pg")
            nc.tensor.matmul(pz[:], wm1[:], rh[:])
            z1 = sgp.tile([128, 128], F32, tag="sg")
            nc.scalar.activation(z1[:], pz[:], mybir.ActivationFunctionType.Tanh)
            po = pgp.tile([4, BS], F32, tag="pg")
            nc.tensor.matmul(po[:], wl2t[:], z1[:])
            o4 = cpool.tile([4, BS], F32)
            nc.scalar.add(o4[:], po[:], bl2[:, 0:1])
            nc.sync.dma_start(out=outd[:], in_=o4[:])

    _split_excess_waits(nc)
    return nc


def pack_weights(Wih, Whh, bias):
    """-> (w_fi, w_go) [128, 128] f32 each.

    Row layout (contraction/features): 0:50 Wih.T, 50 bias, 64:128 Whh.T.
    Column layout: w_fi = [f | i], w_go = [2g | o] (g doubled: sigma(2g) trick).
    Source gate order (torch LSTM): i, f, g, o blocks of 64.
    """
    Wih = np.asarray(Wih, np.float32)  # [256, 50]
    Whh = np.asarray(Whh, np.float32)  # [256, 64]
    bias = np.asarray(bias, np.float32)  # [256]
    w = np.zeros((128, 256), np.float32)

    def reorder(mat):  # mat [*, 256] in i,f,g,o -> [i | f | 2g | o]
        i, f, g, o = mat[:, 0:64], mat[:, 64:128], mat[:, 128:192], mat[:, 192:256]
        return np.concatenate([i, f, 2.0 * g, o], axis=1)

    w[0:D, :] = reorder(Wih.T)
    w[50, :] = reorder(bias[None, :])[0]
    w[64:128, :] = reorder(Whh.T)
    return np.ascontiguousarray(w[:, 0:128]), np.ascontiguousarray(w[:, 128:256])


def build_masks(slen, t_steps):
    """slen: [BS, 64] int -> [128, t_steps*128] u8.

    Rows 0:64 = c-mask, rows 64:128 = h-mask (identical content);
    mask[j, t*128 + b] = (slen[b, j] == t). h_only zeroes rows 0:64.
    """
    eq = (slen.T[:, None, :] == np.arange(t_steps, dtype=slen.dtype)[None, :, None])
    eq = eq.astype(np.uint8)  # [64, t, BS]
    m = np.concatenate([np.zeros_like(eq), eq], axis=0)  # [128, t, BS]
    return np.ascontiguousarray(m.reshape(128, -1))


_prog_cache = {}


def get_program(t1=T1, t2=T2):
    key = (t1, t2)
    if key not in _prog_cache:
        _prog_cache[key] = build_program(t1, t2)
    return _prog_cache[key]


def make_in_maps(sentence1, sentence2, s1_len, s2_len, emb,
                 Wih1, Whh1, bih1, bhh1, Wih2, Whh2, bih2, bhh2,
                 Wl1, bl1, Wl2, bl2, t1=T1, t2=T2):
    emb_pad = np.zeros((V, E), np.float32)
    emb_pad[:, :D] = np.asarray(emb, np.float32)
    emb_pad[:, D] = 1.0  # bias row rides through the gather (row 50 of x.T)
    wfi1, wgo1 = pack_weights(Wih1, Whh1, np.asarray(bih1) + np.asarray(bhh1))
    wfi2, wgo2 = pack_weights(Wih2, Whh2, np.asarray(bih2) + np.asarray(bhh2))
    wm1 = np.zeros((65, 128), np.float32)
    wm1[0:64, :] = np.asarray(Wl1, np.float32).T
    wm1[64, :] = np.asarray(bl1, np.float32)
    wl2t = np.ascontiguousarray(np.asarray(Wl2, np.float32).T)
    bl2c = np.asarray(bl2, np.float32).reshape(4, 1)
    ident = np.eye(128, dtype=np.float32)

    s1t = np.asarray(sentence1, np.int32)
    s2t = np.asarray(sentence2, np.int32)
    l1 = np.asarray(s1_len, np.int64)[:, 0, :]  # [B, 64]
    l2 = np.asarray(s2_len, np.int64)[:, 0, :]

    in_maps = []
    for c in range(NCORES):
        sl = slice(c * BS, (c + 1) * BS)
        in_maps.append({
            "tbl": emb_pad,
            "s1": np.ascontiguousarray(s1t[sl, :t1]),
            "s2": np.ascontiguousarray(s2t[sl, :t2]),
            "wfi1": wfi1, "wgo1": wgo1, "wfi2": wfi2, "wgo2": wgo2,
            "m1": build_masks(l1[sl], t1),
            "m2": build_masks(l2[sl], t2),
            "ident": ident, "wm1": wm1, "wl2t": wl2t, "bl2": bl2c,
        })
    return in_maps


def kernel(sentence1, sentence2, s1_len, s2_len, s1_s, s2_s, emb,
           Wih1, Whh1, bih1, bhh1, Wih2, Whh2, bih2, bhh2,
           Wl1, bl1, Wl2, bl2):
    nc = get_program()
    in_maps = make_in_maps(sentence1, sentence2, s1_len, s2_len, emb,
                           Wih1, Whh1, bih1, bhh1, Wih2, Whh2, bih2, bhh2,
                           Wl1, bl1, Wl2, bl2)
    res = run_bass_kernel_spmd(nc, in_maps, list(range(NCORES)))
    out = np.zeros((B, 4), np.float32)
    for c in range(NCORES):
        out[c * BS : (c + 1) * BS, :] = res.results[c]["out"].T
    return out
